# revision 19
# baseline (speedup 1.0000x reference)
"""Trainium2 kernel for the NNUE-style factorized embedding segment-sum.

Strategy: the ragged two-table embedding-bag is reformulated as block-diagonal
dense matmuls.  For each output row (bag), the gather+segment-sum over its
ragged feature ids equals  counts_row @ table_block, where table_block is the
768-row slice of the MERGED table (tiles + (pieces+ranks+files)*mask,
materialized on the host) selected by the bag's king square.  The second
output's counts columns are flip-remapped on the host so only ONE table is
ever needed.

Host (integer work only): build per-bag count rows, group (output,bag) items
by table block, shard blocks over 8 cores.  Device: per 128-item chunk, 6
accumulating matmuls (K=128, M=128, N=256) with fp8e4 counts (exact for small
integer counts) as the stationary operand and the bf16 merged table as the
moving operand, then a clipped PSUM->SBUF drain in bf16, batched 4 chunks per
activation op / output store.

Blocks are assigned to (core, slot) so that each slot's chunk capacity (shared
across cores — the compiled program is SPMD) matches the data tightly.
"""

import numpy as np
import ml_dtypes

import concourse.bass as bass
import concourse.tile as tile
from concourse import bacc, mybir
from concourse.bass_utils import run_bass_kernel_spmd

N_CORES = 8
B = 16384          # bags
KPL = 12           # piece planes
DOUT = 256
PIECE = 768        # KPL * 64
NCHK = 6           # feature chunks per block (768 / 128)
NBLK = 8           # table blocks per core (64 king squares / 8 cores)
QUAD = 4           # chunks per PSUM tile / output store

# ---------------------------------------------------------------------------
# host-side integer prep tables
_sq = np.arange(64)
_PERM = (7 - _sq // 8) * 8 + _sq % 8          # vertical king-square flip
_v = np.arange(PIECE)
_vk, _vr, _vf = _v // 64, (_v % 64) // 8, _v % 8
_FLIP_COL = ((_vk + 6) % 12) * 64 + (7 - _vr) * 8 + _vf

_prog_cache = {}


def _build_program(caps: tuple, cm_fp8: bool):
    """Raw-bass (no TileContext) program for one core, manual semaphores.

    Engines: SP issues count loads, Act issues table loads + output stores,
    DVE does the PE warmup memset and the clip+downcast PSUM drains, PE does
    the matmuls.  ~20 manual semaphores; a ranged gpsimd sem_clear at the end
    resets state for NEFF re-execution (this replaces the TileContext
    epilogue's ~100-instruction per-semaphore sweep).
    """
    nch = sum(caps)
    nc = bacc.Bacc("TRN2", target_bir_lowering=False, debug=False)
    f32 = mybir.dt.float32
    f16 = mybir.dt.float16
    bf16 = mybir.dt.bfloat16
    cdt = mybir.dt.float8e4 if cm_fp8 else f16

    tab = nc.dram_tensor("tab", [128, NBLK * NCHK * DOUT], f16,
                         kind="ExternalInput").ap()
    cm = nc.dram_tensor("cm", [128, nch * NCHK * 128], cdt,
                        kind="ExternalInput").ap()
    out = nc.dram_tensor("out", [128, nch * DOUT], bf16,
                         kind="ExternalOutput").ap()

    cbase = np.concatenate([[0], np.cumsum(caps)]).astype(int)
    slot_of = np.repeat(np.arange(NBLK), caps)

    tt = nc.alloc_sbuf_tensor("tt", [128, NBLK * NCHK * DOUT], f16)
    ct = nc.alloc_sbuf_tensor("ct", [128, nch * NCHK * 128], cdt)
    outt = nc.alloc_sbuf_tensor("outt", [128, nch * DOUT], bf16)
    wsrc = nc.alloc_sbuf_tensor("wsrc", [128, 256], cdt)
    pss = [nc.alloc_psum_tensor(f"ps{i}", [128, QUAD * DOUT], f32)
           for i in range(4)]

    s_tab = [nc.alloc_semaphore(f"s_tab{s}") for s in range(NBLK)]
    s_cm0a = nc.alloc_semaphore("s_cm0a")
    s_cm = [nc.alloc_semaphore(f"s_cm{s}") for s in range(NBLK)]
    s_w = nc.alloc_semaphore("s_w")
    s_grp = nc.alloc_semaphore("s_grp")
    s_clip = nc.alloc_semaphore("s_clip")
    s_out = nc.alloc_semaphore("s_out")

    tw, w = NCHK * DOUT, NCHK * 128

    # SP: count loads in consumption order (slot0 chunk0 alone for fast fill)
    nc.sync.dma_start(ct[:, :w], cm[:, :w]).then_inc(s_cm0a, 16)
    nc.sync.dma_start(ct[:, w:cbase[1] * w],
                      cm[:, w:cbase[1] * w]).then_inc(s_cm[0], 16)
    for s in range(1, NBLK):
        nc.sync.dma_start(ct[:, cbase[s] * w:cbase[s + 1] * w],
                          cm[:, cbase[s] * w:cbase[s + 1] * w]
                          ).then_inc(s_cm[s], 16)

    # Act: table loads
    for s in range(NBLK):
        nc.scalar.dma_start(tt[:, s * tw:(s + 1) * tw],
                            tab[:, s * tw:(s + 1) * tw]).then_inc(s_tab[s], 16)

    # DVE: warmup source, then per-group clip+downcast drains
    nc.vector.memset(wsrc[:], 0).then_inc(s_w, 1)

    # PE: p-state warmup on the zeroed scratch (PSUM buffer 3; in-order PE
    # stream means group 3's start=True reset makes this safe without sync)
    nc.tensor.wait_ge(s_w, 1)
    for _ in range(16):
        nc.tensor.matmul(pss[3][:, :DOUT], lhsT=wsrc[:, :128], rhs=wsrc[:],
                         start=True, stop=True)

    groups, rem = [], nch
    while rem > 4:
        groups.append(4)
        rem -= 4
    groups += {4: [2, 1, 1], 3: [2, 1], 2: [1, 1], 1: [1]}[rem]

    # store plan: (threshold group count, chunk range) per store
    stores = []
    done = stored = 0
    for gi, k in enumerate(groups):
        done += k
        if done - stored >= 12 or k < 4 or gi == len(groups) - 1:
            stores.append((gi + 1, stored, done))
            stored = done

    # PE main stream + DVE clip stream (emission interleaved; each engine's
    # own order is what matters)
    done = 0
    seen_slot = -1
    for gi, k in enumerate(groups):
        ps = pss[gi % 4]
        if gi >= 4:
            nc.tensor.wait_ge(s_clip, gi - 3)   # PSUM buffer free
        for qi in range(k):
            g = done + qi
            s = slot_of[g]
            if s > seen_slot:                   # new slot: wait its loads
                nc.tensor.wait_ge(s_tab[s], 16)
                nc.tensor.wait_ge(s_cm0a if s == 0 else s_cm[s], 16)
                seen_slot = s
            if s == 0 and g == 1:               # rest of slot 0's counts
                nc.tensor.wait_ge(s_cm[0], 16)
            for j in range(NCHK):
                mm = nc.tensor.matmul(
                    ps[:, qi * DOUT:(qi + 1) * DOUT],
                    lhsT=ct[:, (g * NCHK + j) * 128:(g * NCHK + j + 1) * 128],
                    rhs=tt[:, (s * NCHK + j) * DOUT:(s * NCHK + j + 1) * DOUT],
                    start=(j == 0),
                    stop=(j == NCHK - 1),
                )
        mm.then_inc(s_grp, 1)

        nc.vector.wait_ge(s_grp, gi + 1)
        nc.vector.tensor_scalar(
            outt[:, done * DOUT:(done + k) * DOUT], ps[:, :k * DOUT],
            1.0, 0.0, mybir.AluOpType.min, mybir.AluOpType.max
        ).then_inc(s_clip, 1)
        done += k

    # Act: stores as clip groups complete
    for thr, a, b in stores:
        nc.scalar.wait_ge(s_clip, thr)
        nc.scalar.dma_start(out[:, a * DOUT:b * DOUT],
                            outt[:, a * DOUT:b * DOUT]).then_inc(s_out, 16)
    nc.scalar.wait_ge(s_out, 16 * len(stores))

    # quiesce, then reset every manual semaphore so the NEFF can re-execute
    nc.all_engine_barrier()
    my_sems = s_tab + s_cm + [s_cm0a, s_w, s_grp, s_clip, s_out]
    nc.clear_and_free_semaphores(my_sems)
    nc.all_engine_barrier()

    nc.compile()
    return nc


def _prep(values, lengths, kings):
    """Host prep: per-bag counts and the per-core item layout."""
    values = np.asarray(values).astype(np.int64)
    lengths = np.asarray(lengths).astype(np.int64)
    kings = np.asarray(kings).astype(np.int64)

    seg = np.repeat(np.arange(B, dtype=np.int64), lengths)

    # counts in merged-table column space; output b columns are flip-remapped
    cnt = np.zeros((2 * B + 1, PIECE), np.float32)  # last row stays zero (pad)
    cnt[:B] = np.bincount(seg * PIECE + values,
                          minlength=B * PIECE).reshape(B, PIECE)
    cnt[B:2 * B] = np.bincount(seg * PIECE + _FLIP_COL[values],
                               minlength=B * PIECE).reshape(B, PIECE)

    # block id per (output,bag) item, in merged-table space
    blk = np.concatenate([kings[:, 0], _PERM[kings[:, 1]]])

    order = np.argsort(blk, kind="stable")
    nper = np.bincount(blk, minlength=64)
    offs = np.concatenate([[0], np.cumsum(nper)])
    nchunks = np.maximum(np.ceil(nper / 128).astype(int), 1)

    # assign blocks to (core, slot): sort by descending chunk need so each
    # slot's shared capacity is tight
    rank = np.argsort(-nchunks, kind="stable")      # block ids, desc need
    caps = tuple(int(nchunks[rank[s * N_CORES]]) for s in range(NBLK))
    cbase = np.concatenate([[0], np.cumsum(caps)]).astype(int)
    nch = int(cbase[-1])

    pad_idx = np.full((N_CORES, nch * 128), -1, np.int64)
    for s in range(NBLK):
        for c in range(N_CORES):
            t = rank[s * N_CORES + c]               # block for (core c, slot s)
            ids = order[offs[t]:offs[t + 1]]
            base = cbase[s] * 128
            pad_idx[c, base:base + len(ids)] = ids

    # block index (0..63) per (core, slot), for table selection
    blk_of = rank.reshape(NBLK, N_CORES).T          # [core, slot]

    cm_fp8 = cnt.max() <= 16.0  # fp8e4m3 is exact for integers <= 16
    return cnt, pad_idx, caps, blk_of, cm_fp8


def _make_tab(pieces, ranks, files, tiles, mask, blk_of):
    """Per-core [128, NBLK*NCHK*DOUT] bf16 merged-table slices."""
    pieces = np.asarray(pieces, np.float32)         # [64, K, 1, 1, D]
    ranks = np.asarray(ranks, np.float32)           # [64, K, 8, 1, D]
    files = np.asarray(files, np.float32)           # [64, K, 1, 8, D]
    tiles = np.asarray(tiles, np.float32)           # [64, K, 8, 8, D]
    mask = np.asarray(mask, np.float32)             # [64, K, 8, 8, 1]

    merged = tiles + (pieces + ranks + files) * mask
    planes = merged.reshape(64, PIECE, DOUT).astype(np.float16)
    planes = planes.reshape(64, NCHK, 128, DOUT)

    tabs = []
    for c in range(N_CORES):
        t = planes[blk_of[c]]                  # [NBLK, NCHK, 128, DOUT]
        t = t.transpose(2, 0, 1, 3)            # [128, blk, chunk, dout]
        tabs.append(np.ascontiguousarray(t.reshape(128, -1)))
    return tabs


def _run(inputs, trace=False):
    cnt, pad_idx, caps, blk_of, cm_fp8 = _prep(
        inputs["values"], inputs["lengths"], inputs["kings"])
    nch = sum(caps)
    key = (caps, cm_fp8)
    if key not in _prog_cache:
        _prog_cache[key] = _build_program(caps, cm_fp8)
    nc = _prog_cache[key]

    tabs = _make_tab(inputs["pieces"], inputs["ranks"], inputs["files"],
                     inputs["tiles"], inputs["factorization_mask"], blk_of)

    cdt = ml_dtypes.float8_e4m3 if cm_fp8 else np.float16
    in_maps = []
    for c in range(N_CORES):
        sel = cnt[pad_idx[c]]                  # [nch*128, 768] f32
        cmh = sel.reshape(nch, 128, NCHK, 128).transpose(3, 0, 2, 1)
        in_maps.append({
            "tab": tabs[c],
            "cm": np.ascontiguousarray(cmh.reshape(128, -1).astype(cdt)),
        })

    res = run_bass_kernel_spmd(nc, in_maps, list(range(N_CORES)),
                               trace=trace)

    comb = np.zeros((2 * B, DOUT), np.float32)
    for c in range(N_CORES):
        flat = (res.results[c]["out"].astype(np.float32)
                .reshape(128, nch, DOUT).transpose(1, 0, 2)
                .reshape(nch * 128, DOUT))
        valid = pad_idx[c] >= 0
        comb[pad_idx[c][valid]] = flat[valid]
    return (comb[:B], comb[B:]), res


def kernel(**inputs):
    (a, b), _ = _run(inputs, trace=False)
    return a, b
